# revision 2
# baseline (speedup 1.0000x reference)
"""MultiHeadAttention (B=1, L=4096, D=768, H=12) on 8 trn2 NeuronCores.

Sharding: data-parallel over query positions (L/8 = 512 queries per core).
Each core computes the full K/V projections (replicated; cheaper than any
cross-core collective on this platform), its slice of the Q projection,
attention for all 12 heads over its 512 queries, and the output projection
for its slice. No collectives; host concatenates the 8 output shards.

Layouts (all per core):
  - kp.T [768, 4096] bf16 in SBUF: scores lhsT comes straight from it.
  - scores computed transposed [kpos, q] so AV can contract over kpos.
  - vp [4096, 768] bf16 spilled to DRAM scratch, re-read per head with a
    ones column appended -> AV psum row 64 accumulates the softmax
    denominator for free.
  - softmax has no max-subtraction (scores ~ N(0,1): exp cannot overflow)
    and bk is dropped entirely (constant along the softmax axis).
  - bv and bo are folded into one output-side bias cb = Wo @ bv + bo.
"""

import numpy as np

import concourse.bass as bass
import concourse.bacc as bacc
import concourse.tile as tile
import concourse.mybir as mybir
from concourse.bass_utils import run_bass_kernel_spmd

P = 128
D_MODEL = 768
NUM_HEADS = 12
D_K = 64
NE = D_MODEL // P  # 6 tiles of the model dim

F32 = mybir.dt.float32
F32R = mybir.dt.float32r
BF16 = mybir.dt.bfloat16
Act = mybir.ActivationFunctionType


def build_program(L, LQ, n_cores):
    """Build + compile the per-core Bass program.

    L: total sequence length (keys/values), LQ: queries per core.
    """
    KT = L // P    # kpos chunks of 128 (scores stationary / AV contraction)
    LC = L // 512  # 512-wide l chunks for the kp.T projection
    QT = LQ // P   # query tiles of 128

    nc = bacc.Bacc("TRN2", target_bir_lowering=False, debug=False,
                   num_devices=n_cores)

    qT = nc.dram_tensor("qT", [D_MODEL, LQ], F32, kind="ExternalInput").ap()
    kT = nc.dram_tensor("kT", [D_MODEL, L], F32, kind="ExternalInput").ap()
    vT = nc.dram_tensor("vT", [D_MODEL, L], F32, kind="ExternalInput").ap()
    WqT = nc.dram_tensor("WqT", [D_MODEL, D_MODEL], F32, kind="ExternalInput").ap()
    WkT = nc.dram_tensor("WkT", [D_MODEL, D_MODEL], F32, kind="ExternalInput").ap()
    WvT = nc.dram_tensor("WvT", [D_MODEL, D_MODEL], F32, kind="ExternalInput").ap()
    WoT = nc.dram_tensor("WoT", [D_MODEL, D_MODEL], F32, kind="ExternalInput").ap()
    bq_r = nc.dram_tensor("bq_r", [P, NE], F32, kind="ExternalInput").ap()
    cb_bc = nc.dram_tensor("cb_bc", [P, D_MODEL], F32, kind="ExternalInput").ap()
    ones64 = nc.dram_tensor("ones64", [1, 64], F32, kind="ExternalInput").ap()
    out = nc.dram_tensor("out", [LQ, D_MODEL], F32, kind="ExternalOutput").ap()

    with tile.TileContext(nc) as tc:
        with (
            tc.tile_pool(name="persist", bufs=1) as persist,
            tc.tile_pool(name="dram", bufs=1, space="DRAM") as dram,
            tc.tile_pool(name="bigw", bufs=2) as bigw,      # Wq/Wk, then vh_aug
            tc.tile_pool(name="kt", bufs=8) as kt_pool,     # kT moving tiles
            tc.tile_pool(name="vt", bufs=6) as vt_pool,     # vT stationary tiles
            tc.tile_pool(name="stage", bufs=4) as stage,    # psum->dram staging
            tc.tile_pool(name="exp", bufs=4) as exp_pool,
            tc.tile_pool(name="small", bufs=3) as small,
            tc.tile_pool(name="outst", bufs=4) as outst,
            tc.tile_pool(name="psA", bufs=3, space="PSUM") as psA,   # proj + o-proj
            tc.tile_pool(name="psS", bufs=2, space="PSUM") as psS,   # scores + bcast
            tc.tile_pool(name="psV", bufs=2, space="PSUM") as psV,   # AV accum
        ):
            # ---- persistent SBUF tensors ----
            kpT_sb = persist.tile([P, NE, L], BF16)          # kp.T
            qpT_sb = persist.tile([P, NE, LQ], BF16)         # qp.T
            attnT_sb = persist.tile([P, NE, LQ], F32)        # normalized attn.T
            qT_sb = persist.tile([P, NE, LQ], F32R)
            WvT_sb = persist.tile([P, NE, D_MODEL], F32R)
            WoT_sb = persist.tile([P, NE, D_MODEL], F32R)
            bq_sb = persist.tile([P, NE], F32)
            cb_sb = persist.tile([P, D_MODEL], F32)
            ones_sb = persist.tile([1, 64], F32)

            vp_d = dram.tile([L, D_MODEL], BF16)             # vp spill

            nc.sync.dma_start(out=qT_sb[:], in_=qT.rearrange(
                "(t p) l -> p t l", p=P).bitcast(F32R))
            nc.sync.dma_start(out=WvT_sb[:], in_=WvT.rearrange(
                "(t p) e -> p t e", p=P).bitcast(F32R))
            nc.sync.dma_start(out=WoT_sb[:], in_=WoT.rearrange(
                "(t p) e -> p t e", p=P).bitcast(F32R))
            nc.sync.dma_start(out=bq_sb[:], in_=bq_r)
            nc.sync.dma_start(out=cb_sb[:], in_=cb_bc)
            nc.sync.dma_start(out=ones_sb[:], in_=ones64)

            Wq_all = bigw.tile([P, NE, D_MODEL], F32R, tag="bigw")
            Wk_all = bigw.tile([P, NE, D_MODEL], F32R, tag="bigw")
            nc.sync.dma_start(out=Wq_all[:], in_=WqT.rearrange(
                "(t p) e -> p t e", p=P).bitcast(F32R))
            nc.sync.dma_start(out=Wk_all[:], in_=WkT.rearrange(
                "(t p) e -> p t e", p=P).bitcast(F32R))

            # ---- P1a: qp.T [e, lq] = sum_d WqT[d, e].T @ qT[d, lq], + bq ----
            for e in range(NE):
                ps = psA.tile([P, 512], F32, name="proj")
                for d in range(NE):
                    nc.tensor.matmul(
                        ps[:, :LQ],
                        Wq_all[:, d, e * P:(e + 1) * P],
                        qT_sb[:, d, :],
                        start=(d == 0), stop=(d == NE - 1),
                    )
                nc.scalar.activation(
                    qpT_sb[:, e, :], ps[:, :LQ], Act.Identity,
                    bias=bq_sb[:, e:e + 1],
                )

            # ---- P1b: kp.T [e, l] (bk dropped: softmax-shift invariant) ----
            for l in range(LC):
                kt_tiles = []
                for d in range(NE):
                    t = kt_pool.tile([P, 512], F32R, tag="kt")
                    nc.sync.dma_start(
                        out=t[:],
                        in_=kT[d * P:(d + 1) * P, l * 512:(l + 1) * 512].bitcast(F32R),
                    )
                    kt_tiles.append(t)
                for e in range(NE):
                    ps = psA.tile([P, 512], F32, name="proj")
                    for d in range(NE):
                        nc.tensor.matmul(
                            ps[:],
                            Wk_all[:, d, e * P:(e + 1) * P],
                            kt_tiles[d][:],
                            start=(d == 0), stop=(d == NE - 1),
                        )
                    nc.vector.tensor_copy(
                        out=kpT_sb[:, e, l * 512:(l + 1) * 512], in_=ps[:])

            # ---- P1c: vp [l, e] = vT[:, l].T @ WvT (bv folded into cb) ----
            for lt in range(L // P):
                ps1 = psA.tile([P, 512], F32, name="proj")
                ps2 = psA.tile([P, 512], F32, name="proj")[:, :256]
                for d in range(NE):
                    t = vt_pool.tile([P, P], F32R, tag="vt")
                    nc.sync.dma_start(
                        out=t[:],
                        in_=vT[d * P:(d + 1) * P, lt * P:(lt + 1) * P].bitcast(F32R),
                    )
                    nc.tensor.matmul(ps1[:], t[:], WvT_sb[:, d, 0:512],
                                     start=(d == 0), stop=(d == NE - 1))
                    nc.tensor.matmul(ps2[:], t[:], WvT_sb[:, d, 512:768],
                                     start=(d == 0), stop=(d == NE - 1))
                s1 = stage.tile([P, 512], BF16, tag="st1")
                s2 = stage.tile([P, 256], BF16, tag="st2")
                nc.vector.tensor_copy(out=s1[:], in_=ps1[:])
                nc.vector.tensor_copy(out=s2[:], in_=ps2[:])
                nc.sync.dma_start(out=vp_d[lt * P:(lt + 1) * P, 0:512], in_=s1[:])
                nc.sync.dma_start(out=vp_d[lt * P:(lt + 1) * P, 512:768], in_=s2[:])

            # ---- P2: attention, head by head ----
            vp_view = vp_d[:].rearrange("(c p) e -> p c e", p=P)
            for h in range(NUM_HEADS):
                et, pr = h // 2, (h % 2) * 64
                vh = bigw.tile([P, KT, 65], BF16, tag="bigw", name="vh")
                nc.sync.dma_start(
                    out=vh[:, :, 0:64], in_=vp_view[:, :, h * 64:(h + 1) * 64])
                nc.vector.memset(vh[:, :, 64:65], 1.0)

                qhT = qpT_sb[pr:pr + 64, et, :]
                ps_av = psV.tile([65, 512], F32, name="av")
                for c in range(KT):
                    khT = kpT_sb[pr:pr + 64, et, c * P:(c + 1) * P]
                    ps_s = psS.tile([P, 512], F32, name="sc")
                    nc.tensor.matmul(ps_s[:, :LQ], khT, qhT,
                                     start=True, stop=True)
                    ex = exp_pool.tile([P, 512], BF16, tag="exp")
                    nc.scalar.activation(ex[:, :LQ], ps_s[:, :LQ], Act.Exp,
                                         scale=0.125)
                    nc.tensor.matmul(ps_av[:, :LQ], vh[:, c, :], ex[:, :LQ],
                                     start=(c == 0), stop=(c == KT - 1))

                recip = small.tile([1, 512], F32, tag="recip")
                nc.vector.reciprocal(out=recip[:, :LQ], in_=ps_av[64:65, :LQ])
                ps_bc = psS.tile([64, 512], F32, name="bc", bufs=1)
                nc.tensor.matmul(ps_bc[:, :LQ], ones_sb[:], recip[:, :LQ],
                                 start=True, stop=True)
                rbc = small.tile([64, 512], F32, tag="rbc")
                nc.vector.tensor_copy(out=rbc[:, :LQ], in_=ps_bc[:, :LQ])
                nc.vector.tensor_tensor(
                    out=attnT_sb[pr:pr + 64, et, :],
                    in0=ps_av[0:64, :LQ], in1=rbc[:, :LQ],
                    op=mybir.AluOpType.mult,
                )

            # ---- P3: out[q, e] = attnT.T @ WoT + cb (fp32 matmul: safe) ----
            for qt in range(QT):
                ps1 = psA.tile([P, 512], F32, name="proj")
                ps2 = psA.tile([P, 512], F32, name="proj")[:, :256]
                for d in range(NE):
                    lhs = attnT_sb[:, d, qt * P:(qt + 1) * P]
                    nc.tensor.matmul(ps1[:], lhs, WoT_sb[:, d, 0:512].bitcast(F32),
                                     start=(d == 0), stop=(d == NE - 1))
                    nc.tensor.matmul(ps2[:], lhs, WoT_sb[:, d, 512:768].bitcast(F32),
                                     start=(d == 0), stop=(d == NE - 1))
                o1 = outst.tile([P, 512], F32, tag="o1")
                o2 = outst.tile([P, 256], F32, tag="o2")
                nc.vector.tensor_tensor(out=o1[:], in0=ps1[:], in1=cb_sb[:, 0:512],
                                        op=mybir.AluOpType.add)
                nc.vector.tensor_tensor(out=o2[:], in0=ps2[:], in1=cb_sb[:, 512:768],
                                        op=mybir.AluOpType.add)
                nc.sync.dma_start(out=out[qt * P:(qt + 1) * P, 0:512], in_=o1[:])
                nc.sync.dma_start(out=out[qt * P:(qt + 1) * P, 512:768], in_=o2[:])

    nc.compile()
    return nc


def make_in_maps(q, k, v, Wq, bq, Wk, bk, Wv, bv, Wo, bo, L, LQ, n_cores):
    f = np.float32
    qT_full = np.ascontiguousarray(q[0].T, dtype=f)        # [768, L]
    kT_full = np.ascontiguousarray(k[0].T, dtype=f)
    vT_full = np.ascontiguousarray(v[0].T, dtype=f)
    WqT = np.ascontiguousarray(np.asarray(Wq, f).T)
    WkT = np.ascontiguousarray(np.asarray(Wk, f).T)
    WvT = np.ascontiguousarray(np.asarray(Wv, f).T)
    WoT = np.ascontiguousarray(np.asarray(Wo, f).T)
    bq_r = np.ascontiguousarray(np.asarray(bq, f).reshape(NE, P).T)
    cb = np.asarray(Wo, f) @ np.asarray(bv, f) + np.asarray(bo, f)
    cb_bc = np.ascontiguousarray(np.broadcast_to(cb, (P, D_MODEL)))
    ones = np.ones((1, 64), f)
    shared = dict(kT=kT_full, vT=vT_full, WqT=WqT, WkT=WkT, WvT=WvT,
                  WoT=WoT, bq_r=bq_r, cb_bc=cb_bc, ones64=ones)
    return [
        {"qT": np.ascontiguousarray(qT_full[:, c * LQ:(c + 1) * LQ]), **shared}
        for c in range(n_cores)
    ]


_PROGRAM_CACHE = {}


def get_program(L, LQ, n_cores):
    key = (L, LQ, n_cores)
    if key not in _PROGRAM_CACHE:
        _PROGRAM_CACHE[key] = build_program(L, LQ, n_cores)
    return _PROGRAM_CACHE[key]


def kernel(q, k, v, Wq, bq, Wk, bk, Wv, bv, Wo, bo):
    B, L, _ = q.shape
    assert B == 1
    n_cores = 8
    LQ = L // n_cores
    nc = get_program(L, LQ, n_cores)
    in_maps = make_in_maps(q, k, v, Wq, bq, Wk, bk, Wv, bv, Wo, bo,
                           L, LQ, n_cores)
    res = run_bass_kernel_spmd(nc, in_maps, core_ids=list(range(n_cores)))
    full = np.concatenate([res.results[c]["out"] for c in range(n_cores)], axis=0)
    return full[None].astype(np.float32)
